# revision 14
# baseline (speedup 1.0000x reference)
"""Guide-token attention kernel for Trainium2 (8 NeuronCores).

Module: y[b] = softmax(((Q+tQ) @ (K+tK)^T)/sqrt(hd)) @ V  per head, where
  Q = x @ Wq^T + bq, K = x @ Wk^T + bk, V = x @ Wv^T + bv,
  tQ/tK are projections of a per-batch guide token (broadcast over seq).

Shapes: x [4, 1024, 1024], tokens [4, 1, 1024], W* [1024, 1024], b* [1024].
H=16 heads, hd=64.

Sharding: 8 cores = 4 batches x 2 head-groups (8 heads each); weights
column-sharded per head group; each core sees one batch -> no cross-core
communication.

Design (v7):
  - PE is the bottleneck: ~390 effective N=512 matmul slots x ~220 ns
    (192 proj + 64 row-tiled score pair-slots + 128 AV) ≈ 88 us.  ACT exp
    (71 us) and DVE (~45 us) must hide under the PE stream, so the whole
    schedule optimizes PE density: end ≈ PE_start + PE_total + last-chain.
  - Input DMA on both HW DGE queues (sync + scalar), critical bytes first
    (x + ft0 weights), chunked so matmuls unblock incrementally; first
    score pair needs only the sb0 half of the keys.
  - Warmup matmuls interleave with the DMA-paced projection phase to keep
    the PE HAM clock-gate warm (sustained activity -> 2.4 GHz).
  - Budget-based emission: per score pair (4 MMs + 2 exps ≈ ACT 2.23 us)
    emit ~8 one-slot ops drawn from projection fillers first, then
    trailing AV groups (4 slots each), keeping PE exactly rate-matched
    with no in-order stalls; AV trails scores >=2 pairs so probs are
    always ready.
  - Normalize: den copy -> fast reciprocal -> GpSimd partition-broadcast
    -> one DVE multiply -> bf16 yT; per-128-feature output DMA.
"""

import os

import numpy as np
import ml_dtypes

import concourse.bass as bass
import concourse.tile as tile
from concourse import bacc
from concourse import mybir
from concourse.bass_utils import run_bass_kernel_spmd

B = 4
S = 1024
D = 1024
H = 16
HD = 64
NCORES = 8
FPG = 512          # features per head-group (8 heads * 64)
NKC = D // 128     # contraction chunks for projections
NFT = FPG // 128   # feature tiles per group
NST = S // 128     # sequence tiles
NQB = S // 512     # 512-wide query blocks
HPG = 8            # heads per group
NPAIR = NST // 2   # kt pairs per unit

BF16 = mybir.dt.bfloat16
F32 = mybir.dt.float32

PAIR_BUDGET = 8    # non-score PE slots emitted per score pair
AVQ_CAP = 12       # force AV pops above this backlog (bounds probs pool)

_CACHE = {}


def _build():
    nc = bacc.Bacc()

    # ---- DRAM inputs ----
    # sync queue:   xA0 xB0 qadd kadd wk0 xA1 xB1 wk1       (+ yT out later)
    # scalar queue: wq0 xC0 xD0 wq1 xC1 xD1 wv wq2 wk2 wq3 wk3
    xd = {}
    for cname in ("xA0", "xB0", "xC0", "xD0", "xA1", "xB1", "xC1", "xD1"):
        xd[cname] = nc.declare_dram_parameter(cname, [128, 2, 512], BF16, isOutput=False)
    wqd = [nc.declare_dram_parameter(f"wq{f}", [128, NKC, 128], BF16, isOutput=False)
           for f in range(NFT)]
    wkd = [nc.declare_dram_parameter(f"wk{f}", [128, NKC, 128], BF16, isOutput=False)
           for f in range(NFT)]
    wvd = nc.declare_dram_parameter("wv", [128, NKC, FPG], BF16, isOutput=False)
    qaddd = nc.declare_dram_parameter("qadd", [128, NFT], F32, isOutput=False)
    kaddd = nc.declare_dram_parameter("kadd", [128, NFT], F32, isOutput=False)
    yTd = nc.declare_dram_parameter("yT", [NFT, 128, S], BF16, isOutput=True)

    with tile.TileContext(nc) as tc:
        with (
            tc.tile_pool(name="persist", bufs=1) as persist,
            tc.tile_pool(name="probs", bufs=36) as probs_pool,
            tc.tile_pool(name="norm", bufs=4) as norm_pool,
            tc.tile_pool(name="psP", bufs=2, space=bass.MemorySpace.PSUM) as psP,
            tc.tile_pool(name="psA", bufs=2, space=bass.MemorySpace.PSUM) as psA,
            tc.tile_pool(name="psAV", bufs=2, space=bass.MemorySpace.PSUM) as psAV,
        ):
            # ---- persistent SBUF tensors (chunked to DMA granularity) ----
            xts = {(c, h): persist.tile([128, 2, 512], BF16, name=f"xt{c}{h}")
                   for c in range(4) for h in range(2)}
            wq_sb = [persist.tile([128, NKC, 128], BF16, name=f"wqs{f}")
                     for f in range(NFT)]
            wk_sb = [persist.tile([128, NKC, 128], BF16, name=f"wks{f}")
                     for f in range(NFT)]
            wv_sb = persist.tile([128, NKC, FPG], BF16)
            qa = persist.tile([128, NFT], F32)
            ka = persist.tile([128, NFT], F32)
            cq = [persist.tile([128, S], BF16, name=f"cq{i}") for i in range(NFT)]
            ck = [persist.tile([128, S], BF16, name=f"ck{i}") for i in range(NFT)]
            vts = [persist.tile([128, HPG, HD + 1], BF16, name=f"vt{i}")
                   for i in range(NST)]
            yt = persist.tile([128, NFT, S], BF16)
            wrm = persist.tile([128, 512], BF16)

            # ---- input DMAs, critical bytes first on both HW queues ----
            nc.sync.dma_start(out=xts[(0, 0)][:], in_=xd["xA0"][:])
            nc.sync.dma_start(out=xts[(1, 0)][:], in_=xd["xB0"][:])
            nc.sync.dma_start(out=qa[:], in_=qaddd[:])
            nc.sync.dma_start(out=ka[:], in_=kaddd[:])
            nc.sync.dma_start(out=wk_sb[0][:], in_=wkd[0][:])
            nc.sync.dma_start(out=xts[(0, 1)][:], in_=xd["xA1"][:])
            nc.sync.dma_start(out=xts[(1, 1)][:], in_=xd["xB1"][:])
            nc.sync.dma_start(out=wk_sb[1][:], in_=wkd[1][:])
            nc.scalar.dma_start(out=wq_sb[0][:], in_=wqd[0][:])
            nc.scalar.dma_start(out=xts[(2, 0)][:], in_=xd["xC0"][:])
            nc.scalar.dma_start(out=xts[(3, 0)][:], in_=xd["xD0"][:])
            nc.scalar.dma_start(out=wq_sb[1][:], in_=wqd[1][:])
            nc.scalar.dma_start(out=xts[(2, 1)][:], in_=xd["xC1"][:])
            nc.scalar.dma_start(out=xts[(3, 1)][:], in_=xd["xD1"][:])
            nc.scalar.dma_start(out=wv_sb[:], in_=wvd[:])
            nc.scalar.dma_start(out=wq_sb[2][:], in_=wqd[2][:])
            nc.scalar.dma_start(out=wk_sb[2][:], in_=wkd[2][:])
            nc.scalar.dma_start(out=wq_sb[3][:], in_=wqd[3][:])
            nc.scalar.dma_start(out=wk_sb[3][:], in_=wkd[3][:])

            # ones columns for the AV denominator rows + warmup source
            nc.vector.memset(wrm[:], 0.0)
            for st in range(NST):
                nc.vector.memset(vts[st][:, :, HD:HD + 1], 1.0)

            # ---- HAM warmup machinery ----
            wacc = psAV.tile([128, 512], F32, tag="psAV")
            warm_left = [22]

            def warm_mm():
                if warm_left[0] > 0:
                    warm_left[0] -= 1
                    nc.tensor.matmul(
                        wacc[:], wrm[:, 0:128], wrm[:], start=True, stop=True
                    )

            for _ in range(6):
                warm_mm()

            # ---- projection building blocks ----
            v_done = [0]      # V groups fully emitted (gates AV emission)
            qk_done = set()   # (which, ft, sb) evictions emitted

            def qk_group(which, ft, sb):
                """QT/KT [128 feat, 512 q] accumulated over D chunks, evicted
                to bf16 with the guide-token add (+1/8 scale folded into Q).
                Yields (slots, op)."""
                if which == "q":
                    w_sb, add_sb, scale, dst = wq_sb[ft], qa, 0.125, cq[ft]
                else:
                    w_sb, add_sb, scale, dst = wk_sb[ft], ka, 1.0, ck[ft]
                acc = psP.tile([128, 512], F32, tag="psP")
                for kc in range(NKC):
                    yield 1, lambda kc=kc, acc=acc: nc.tensor.matmul(
                        acc[:],
                        w_sb[:, kc, :],
                        xts[(kc // 2, sb)][:, kc % 2, :],
                        start=(kc == 0),
                        stop=(kc == NKC - 1),
                    )

                def evict(acc=acc):
                    nc.vector.tensor_scalar(
                        out=dst[:, sb * 512:(sb + 1) * 512],
                        in0=acc[:],
                        scalar1=scale,
                        scalar2=add_sb[:, ft:ft + 1],
                        op0=mybir.AluOpType.mult,
                        op1=mybir.AluOpType.add,
                    )
                    qk_done.add((which, ft, sb))

                yield 0, evict

            def v_group(st):
                """V [128 seq, 512 feat] natural layout, strided into vts."""
                acc = psP.tile([128, 512], F32, tag="psP")
                for kc in range(NKC):
                    yield 1, lambda kc=kc, acc=acc: nc.tensor.matmul(
                        acc[:],
                        xts[(kc // 2, st // 4)][:, kc % 2,
                                                (st % 4) * 128:(st % 4 + 1) * 128],
                        wv_sb[:, kc, :],
                        start=(kc == 0),
                        stop=(kc == NKC - 1),
                    )

                def evict(acc=acc):
                    nc.vector.tensor_copy(out=vts[st][:, :, 0:HD], in_=acc[:])
                    v_done[0] += 1

                yield 0, evict

            def filler_stream():
                yield from qk_group("k", 0, 1)      # keys sb1 (unit0 p2/p3)
                yield from qk_group("q", 0, 1)      # unit (0,1)
                yield from qk_group("q", 1, 0)
                yield from qk_group("k", 1, 0)
                yield from qk_group("k", 1, 1)
                yield from v_group(0)
                yield from v_group(1)
                yield from v_group(2)
                yield from v_group(3)
                yield from qk_group("q", 1, 1)
                for st in range(4, NST):
                    yield from v_group(st)
                yield from qk_group("q", 2, 0)
                yield from qk_group("k", 2, 0)
                yield from qk_group("k", 2, 1)
                yield from qk_group("q", 2, 1)
                yield from qk_group("q", 3, 0)
                yield from qk_group("k", 3, 0)
                yield from qk_group("k", 3, 1)
                yield from qk_group("q", 3, 1)

            # ---- output flush tracking ----
            done_units = set()

            def maybe_flush(hp, qb):
                done_units.add((hp, qb))
                if all((hp, q) in done_units for q in range(NQB)):
                    nc.sync.dma_start(out=yTd[hp], in_=yt[:, hp, :])

            # ---- AV + normalize ----
            av_tiles = {}

            def av_ops(u, p, pairs):
                hp, qb = UNITS[u]
                if p == 0:
                    av_tiles[u] = (
                        psAV.tile([HD + 1, 512], F32, tag="psAV", name=f"av{u}e"),
                        psAV.tile([HD + 1, 512], F32, tag="psAV", name=f"av{u}o"),
                    )
                av_e, av_o = av_tiles[u]
                prA, prB = pairs[p]
                for j in range(2):
                    kt = 2 * p + j
                    nc.tensor.matmul(
                        av_e[:], vts[kt][:, 2 * hp, :], prA[:, j, :],
                        start=(kt == 0), stop=(kt == NST - 1),
                    )
                    nc.tensor.matmul(
                        av_o[:], vts[kt][:, 2 * hp + 1, :], prB[:, j, :],
                        start=(kt == 0), stop=(kt == NST - 1),
                    )
                if p == NPAIR - 1:
                    qsl = slice(qb * 512, (qb + 1) * 512)
                    for h_i, av in ((0, av_e), (1, av_o)):
                        den = norm_pool.tile([1, 512], F32, tag="den")
                        nc.vector.tensor_copy(out=den[:], in_=av[HD:HD + 1, :])
                        rec = norm_pool.tile([1, 512], F32, tag="rec")
                        nc.vector.reciprocal_approx_fast(out=rec[:], in_=den[:])
                        recb = norm_pool.tile([HD, 512], F32, tag="recb")
                        nc.gpsimd.partition_broadcast(recb[:], rec[:])
                        nc.vector.tensor_tensor(
                            out=yt[h_i * 64:h_i * 64 + 64, hp, qsl],
                            in0=av[0:HD, :],
                            in1=recb[:],
                            op=mybir.AluOpType.mult,
                        )
                    del av_tiles[u]
                    maybe_flush(hp, qb)

            # ---- phase 0: projections for unit (0,0) pair 0, HAM-padded ----
            for slots, op in qk_group("q", 0, 0):
                op()
                if slots:
                    warm_mm()
            for slots, op in qk_group("k", 0, 0):
                op()
                if slots:
                    warm_mm()

            UNITS = [(hp, qb) for hp in range(HPG // 2) for qb in range(NQB)]
            fillers = filler_stream()
            fillers_done = [False]
            avq = []
            pairs_of = {}

            def next_filler():
                item = next(fillers, None)
                if item is None:
                    fillers_done[0] = True
                    return None
                return item

            def pop_av():
                """Emit the oldest pending AV group if allowed; 4 PE slots."""
                if not avq:
                    return 0
                au, ap_ = avq[0]
                if v_done[0] < 2 * ap_ + 2:
                    return 0
                avq.pop(0)
                av_ops(au, ap_, pairs_of[au])
                return 4

            def pair_ready(hp, qb, p):
                return ("q", hp, qb) in qk_done and ("k", hp, p // 2) in qk_done

            def emit_pair(u, p):
                hp, qb = UNITS[u]
                qsl = slice(qb * 512, (qb + 1) * 512)
                scA = psA.tile([128, 2, 512], F32, tag="psA")
                scB = psA.tile([128, 2, 512], F32, tag="psA")
                for j in range(2):
                    kt = 2 * p + j
                    ksl = slice(kt * 128, (kt + 1) * 128)
                    nc.tensor.matmul(
                        scA[:, j, :], ck[hp][0:64, ksl], cq[hp][0:64, qsl],
                        start=True, stop=True,
                    )
                    nc.tensor.matmul(
                        scB[:, j, :], ck[hp][64:128, ksl], cq[hp][64:128, qsl],
                        start=True, stop=True,
                    )
                prA = probs_pool.tile([128, 2, 512], BF16, tag="probs")
                nc.scalar.activation(
                    out=prA[:], in_=scA[:],
                    func=mybir.ActivationFunctionType.Exp,
                )
                prB = probs_pool.tile([128, 2, 512], BF16, tag="probs")
                nc.scalar.activation(
                    out=prB[:], in_=scB[:],
                    func=mybir.ActivationFunctionType.Exp,
                )
                pairs_of[u].append((prA, prB))
                avq.append((u, p))

            for u in range(len(UNITS)):
                pairs_of[u] = []
                for p in range(NPAIR):
                    # correctness: projections this pair reads must be
                    # in-stream before its score matmuls
                    while not pair_ready(*UNITS[u], p):
                        item = next_filler()
                        if item is None:
                            break
                        item[1]()
                    emit_pair(u, p)
                    # rate-match ACT (~2.23us/pair): ~8 non-score PE slots.
                    # Backlog cap forces AV drainage so probs pool stays
                    # bounded; otherwise prefer projection fillers, then AV.
                    budget = PAIR_BUDGET
                    if len(avq) > AVQ_CAP:
                        budget -= pop_av()
                        budget -= pop_av()
                    while budget > 0:
                        if not fillers_done[0]:
                            item = next_filler()
                            if item is not None:
                                budget -= item[0]
                                item[1]()
                                continue
                        if len(avq) > 2:
                            got = pop_av()
                            if got:
                                budget -= got
                                continue
                        break

            # drain: remaining fillers, then trailing AV groups
            while True:
                item = next_filler()
                if item is None:
                    break
                item[1]()
            while avq:
                au, ap_ = avq.pop(0)
                av_ops(au, ap_, pairs_of[au])

    nc.finalize()
    return nc


def _get_nc():
    if "nc" not in _CACHE:
        _CACHE["nc"] = _build()
    return _CACHE["nc"]


def kernel(x, tokens, Wq, bq, Wk, bk, Wv, bv):
    x = np.asarray(x, dtype=np.float32)
    tokens = np.asarray(tokens, dtype=np.float32)
    Wq = np.asarray(Wq, dtype=np.float32)
    Wk = np.asarray(Wk, dtype=np.float32)
    Wv = np.asarray(Wv, dtype=np.float32)
    bq = np.asarray(bq, dtype=np.float32)
    bk = np.asarray(bk, dtype=np.float32)
    bv = np.asarray(bv, dtype=np.float32)

    bf16 = ml_dtypes.bfloat16
    in_maps = []
    for c in range(NCORES):
        b, g = divmod(c, 2)
        rows = slice(g * FPG, (g + 1) * FPG)
        tq = tokens[b, 0] @ Wq[rows].T + 2.0 * bq[rows]   # [512]
        tk = tokens[b, 0] @ Wk[rows].T + 2.0 * bk[rows]

        def packw(aT):
            # [D, C] -> [128, NKC, C] partition-major
            return np.ascontiguousarray(
                aT.reshape(NKC, 128, aT.shape[1]).transpose(1, 0, 2)
            ).astype(bf16)

        xTb = x[b].T.reshape(NKC, 128, S)   # [kc, p, s]
        wqT = Wq[rows].T
        wkT = Wk[rows].T
        m = {
            "wv": packw(Wv[rows].T),
            "qadd": np.ascontiguousarray((tq / 8.0).reshape(NFT, 128).T).astype(np.float32),
            "kadd": np.ascontiguousarray(tk.reshape(NFT, 128).T).astype(np.float32),
        }
        for f in range(NFT):
            m[f"wq{f}"] = packw(wqT[:, f * 128:(f + 1) * 128])
            m[f"wk{f}"] = packw(wkT[:, f * 128:(f + 1) * 128])
        for ci, cl in enumerate("ABCD"):
            xp = xTb[2 * ci:2 * ci + 2].transpose(1, 0, 2)  # [128, 2, 1024]
            m[f"x{cl}0"] = np.ascontiguousarray(xp[:, :, 0:512]).astype(bf16)
            m[f"x{cl}1"] = np.ascontiguousarray(xp[:, :, 512:1024]).astype(bf16)
        in_maps.append(m)

    nc = _get_nc()
    trace = bool(int(os.environ.get("KERNEL_TRACE", "0")))
    res = run_bass_kernel_spmd(nc, in_maps, core_ids=list(range(NCORES)), trace=trace)
    if trace:
        _CACHE["last_results"] = res

    y = np.empty((B, S, D), dtype=np.float32)
    for c in range(NCORES):
        b, g = divmod(c, 2)
        yT = np.asarray(res.results[c]["yT"], dtype=np.float32)  # [4, 128, 1024]
        y[b, :, g * FPG:(g + 1) * FPG] = yT.reshape(FPG, S).T
    y += bv[None, None, :]
    return y
